# revision 18
# baseline (speedup 1.0000x reference)
"""DeepSeek-style local-window sparse attention on 8 TRN2 NeuronCores.

Problem: B=2, N=2048, D=768, H=12 heads x d=64, local window |q-k| <= 64,
out = softmax(mask(q k^T / 8)) v  projected by Wo.

Sharding (no on-device collectives):
  core c in 0..7 -> batch b = c//4, head group g = c%4 (heads 3g..3g+2).
  Each core computes its 3 heads' q/k/v projections over the full
  sequence, the banded attention, and a PARTIAL output projection
  (its 192 rows of Wo). The host sums the 4 partials per batch.

Device layout choices:
  - x is fed pre-transposed (xT [768, 2048]) so projections need no
    on-device transpose: q^T/k^T come out d-major (x^T as moving
    operand), v comes out token-major (x^T as stationary operand).
  - scores are computed transposed (S^T[tk, tq]) so exp(S^T) feeds the
    attn@v matmul directly as the stationary operand; the softmax
    denominator falls out of the same matmul via a ones-column
    appended to v; normalization is a per-partition scalar multiply.
  - softmax skips the running-max: scores*scale here are ~N(0, 0.31),
    so exp never overflows (verified against the reference).
  - bk shifts scores per-query-row only (softmax-invariant) but is
    applied anyway with bq via the free per-partition bias slot of the
    PSUM->SBUF copy. bv and bo pass through the softmax/projection
    linearly and are added on the host: out += bv @ Wo + bo.
  - per 128-query block i the 129-wide window is covered by exactly
    TWO 128-key slots: [128i-64, 128i+64) and [128i+64, 128i+192),
    with fixed triangular masks multiplied into exp(S^T). A second,
    64-token-shifted copy of v (built by two SBUF->SBUF DMAs — only
    DMA can move data across partitions) keeps the attn@v stationary
    operand aligned. Edge blocks swap in an aligned slot + edge mask.

Matmul operands are bf16 (f32 PSUM accumulation); partial outputs are
returned f32 and reduced on the host.
"""

import numpy as np
import ml_dtypes

import concourse.bass as bass
import concourse.tile as tile
from concourse import mybir
from concourse.bass_utils import run_bass_kernel_spmd

BF16 = mybir.dt.bfloat16
F32 = mybir.dt.float32
AF = mybir.ActivationFunctionType

B, N, D = 2, 2048, 768
H, DH = 12, 64
HPC = 3              # heads per core
GC = HPC * DH        # 192 output columns per core
NB = N // 128        # 16 query blocks
KC = D // 128        # 6 contraction chunks
SCALE = 0.125        # 1/sqrt(64)
NCORES = 8
VW = HPC * 65        # 195: v chunk width incl. ones columns


def _split_multiwaits(nc):
    """Hoist extra semaphore waits onto standalone EventSemaphore ops.

    The walrus build in this container rejects any instruction carrying
    more than one sync wait ("Too many sync wait commands"); Tile's
    semaphore assignment freely attaches several. An engine sequencer
    executes instructions in order, so waiting via a standalone
    EventSemaphore then via the instruction itself is equivalent to one
    instruction waiting on both. For DMAs the wait moves from the DGE
    descriptor to the issuing sequencer, which only delays the enqueue.
    """
    n = 0
    for fn in nc.m.functions:
        for bb in fn.blocks:
            out = []
            for inst in bb.instructions:
                si = inst.sync_info
                if si is not None and len(si.on_wait) > 1:
                    extras = list(si.on_wait[:-1])
                    si.on_wait = [si.on_wait[-1]]
                    for w in extras:
                        es = mybir.InstEventSemaphore(
                            name=f"splitw_{n}", ins=[], outs=[]
                        )
                        n += 1
                        es.engine = inst.engine
                        es.sync_info = mybir.SyncInfo(on_wait=[w], on_update=[])
                        nc.register_instruction(es)
                        out.append(es)
                out.append(inst)
            bb.instructions = out


def _emit(nc, tc, d):
    from contextlib import ExitStack

    with ExitStack() as ctx:
        const = ctx.enter_context(tc.tile_pool(name="const", bufs=1))
        persist = ctx.enter_context(tc.tile_pool(name="persist", bufs=1))
        ps_big = ctx.enter_context(tc.tile_pool(name="ps_big", bufs=2, space="PSUM"))
        ps_s = ctx.enter_context(tc.tile_pool(name="ps_s", bufs=3, space="PSUM"))
        ps_sm = ctx.enter_context(tc.tile_pool(name="ps_sm", bufs=3, space="PSUM"))
        e_pool = ctx.enter_context(tc.tile_pool(name="e_pool", bufs=10))
        ao_pool = ctx.enter_context(tc.tile_pool(name="ao_pool", bufs=3))
        o_pool = ctx.enter_context(tc.tile_pool(name="o_pool", bufs=2))
        zr_pool = ctx.enter_context(tc.tile_pool(name="zr_pool", bufs=4))

        # ---- PE warmup: dense dummy matmuls on zeroed SBUF while the
        # input DMAs land, so the HAM clock governor reaches 8/8 before
        # the first real projection instead of ~20us into them ----
        wz = const.tile([128, 512], BF16, name="wz", tag="wz")
        nc.vector.memset(wz[:], 0.0)
        psw = ps_big.tile([128, 512], F32, name="psw", tag="big")
        for _ in range(40):
            nc.tensor.matmul(psw[:], wz[:, 0:128], wz[:], start=True, stop=True)

        # ---- loads: one DMA per 128-partition chunk (parallel queues) ----
        def load_all(name, width):
            t = const.tile([128, KC * width], BF16, name=name, tag=name)
            for k in range(KC):
                nc.sync.dma_start(
                    t[:, width * k : width * (k + 1)],
                    d[name][128 * k : 128 * (k + 1), :],
                )
            return t

        xts_all = load_all("xT", N)
        wqk_all = load_all("wqk", 512)
        wv_all = load_all("wv", GC)
        xts = [xts_all[:, N * k : N * (k + 1)] for k in range(KC)]
        wqk_sb = [wqk_all[:, 512 * k : 512 * (k + 1)] for k in range(KC)]
        wv_sb = [wv_all[:, GC * k : GC * (k + 1)] for k in range(KC)]

        wo0_sb = const.tile([128, D], BF16, name="wo0", tag="wo0")
        nc.sync.dma_start(wo0_sb[:], d["wo0"][:, :])
        wo1_sb = const.tile([64, D], BF16, name="wo1", tag="wo1")
        nc.sync.dma_start(wo1_sb[:], d["wo1"][:, :])
        bqk_sb = const.tile([128, 4], F32, name="bqk", tag="bqk")
        nc.sync.dma_start(bqk_sb[:], d["bqk"][:, :])
        masks_sb = const.tile([128, 3 * 256], BF16, name="masks", tag="masks")
        nc.sync.dma_start(masks_sb[:], d["masks"][:, :])
        ident_sb = const.tile([128, 128], BF16, name="ident", tag="ident")
        nc.sync.dma_start(ident_sb[:], d["ident"][:, :])

        # ---- persistent intermediates ----
        # packed [q0|q1], [q2|pad], [k0|k1], [k2|pad] d-major: padding keeps
        # q_h and k_h at the same partition offset (matmul base-partition
        # rule); offsets 0/64 also alternate PE row groups between heads.
        qk_sb = [
            persist.tile([128, N], BF16, name=f"qk{m}", tag=f"qk{m}") for m in range(4)
        ]
        # v token-major with a ones column per head: per 128-token chunk the
        # 195 cols are [v_h0(64) | 1 | v_h1(64) | 1 | v_h2(64) | 1].
        # v3a: chunks [128j, 128j+128); v3b: shifted chunks [128j+64, ...).
        v3a = persist.tile([128, NB * VW], BF16, name="v3a", tag="v3a")
        v3b = persist.tile([128, (NB - 1) * VW], BF16, name="v3b", tag="v3b")
        ones_ap = v3a[:].rearrange("p (c g s) -> p c g s", g=HPC, s=65)[:, :, :, 64:65]
        nc.vector.memset(ones_ap, 1.0)
        # attention output, head-major (A0: packed rows 0..127, A1: 128..191)
        a0_sb = persist.tile([128, N], BF16, name="a0", tag="a0")
        a1_sb = persist.tile([64, N], BF16, name="a1", tag="a1")

        # ---- q/k projections: qk^T[m] = wqk[:, m-chunk]^T @ x^T ----
        for m in range(4):
            for n in range(N // 512):
                ps = ps_big.tile([128, 512], F32, name=f"psqk{m}_{n}", tag="big")
                for k in range(KC):
                    nc.tensor.matmul(
                        ps[:],
                        wqk_sb[k][:, 128 * m : 128 * (m + 1)],
                        xts[k][:, 512 * n : 512 * (n + 1)],
                        start=(k == 0),
                        stop=(k == KC - 1),
                    )
                nc.scalar.activation(
                    qk_sb[m][:, 512 * n : 512 * (n + 1)],
                    ps[:],
                    AF.Identity,
                    bias=bqk_sb[:, m : m + 1],
                    scale=1.0,
                )

        # ---- v projection (token-major): v[j-chunk] = x^T[:, chunk]^T @ wv ----
        for j in range(NB):
            psv = ps_big.tile([128, GC], F32, name=f"psv{j}", tag="big")
            for k in range(KC):
                nc.tensor.matmul(
                    psv[:],
                    xts[k][:, 128 * j : 128 * (j + 1)],
                    wv_sb[k][:],
                    start=(k == 0),
                    stop=(k == KC - 1),
                )
            vout = v3a[:, VW * j : VW * (j + 1)].rearrange(
                "p (g s) -> p g s", g=HPC, s=65
            )[:, :, 0:64]
            vin = psv[:].rearrange("p (g s) -> p g s", g=HPC, s=DH)
            nc.vector.tensor_copy(vout, vin)
        # shifted copy: v3b[p, j, :] = v tokens 128j+64+p (partition shift
        # needs DMA). Ones columns come along for free.
        v3a_c = v3a[:].rearrange("p (c w) -> p c w", w=VW)
        v3b_c = v3b[:].rearrange("p (c w) -> p c w", w=VW)
        nc.gpsimd.dma_start(v3b_c[0:64, :, :], v3a_c[64:128, 0 : NB - 1, :])
        nc.gpsimd.dma_start(v3b_c[64:128, :, :], v3a_c[0:64, 1:NB, :])

        # ---- banded attention, software-pipelined ----
        # PE stalls on the exp/mask chain if attnV(i) directly follows
        # scores(i): emit scores for block i but attnV/normalize/transpose
        # for block i-PIPE so the PE always has independent matmuls while
        # ACT/DVE fill E. Out-proj quarters are interleaved as soon as
        # their four attention blocks are flushed (PE clock re-throttles
        # when the array idles).
        PIPE = 2
        stage = {}

        def emit_scores(i):
            # two 128-key slots; slot token starts (aligned at the edges)
            s0 = 0 if i == 0 else 128 * i - 64
            s1 = 128 * (NB - 1) if i == NB - 1 else 128 * i + 64
            mvar = 1 if i == 0 else (2 if i == NB - 1 else 0)
            ao3 = ao_pool.tile([128, GC], BF16, name=f"ao{i}", tag="ao")
            heads = []
            for h in range(HPC):
                # S^T[tk, tq] per slot
                pss = ps_s.tile([128, 256], F32, name=f"pss{i}_{h}", tag="s")
                mq, qo = (0, 64 * h) if h < 2 else (1, 0)
                mk, ko = (2, 64 * h) if h < 2 else (3, 0)
                for c, s in enumerate((s0, s1)):
                    nc.tensor.matmul(
                        pss[:, 128 * c : 128 * (c + 1)],
                        qk_sb[mk][ko : ko + 64, s : s + 128],
                        qk_sb[mq][qo : qo + 64, 128 * i : 128 * (i + 1)],
                        start=True,
                        stop=True,
                    )
                e = e_pool.tile([128, 256], BF16, name=f"e{i}_{h}", tag="e")
                nc.scalar.activation(e[:], pss[:], AF.Exp, scale=SCALE)
                nc.vector.tensor_mul(
                    e[:], e[:], masks_sb[:, 256 * mvar : 256 * (mvar + 1)]
                )
                heads.append((h, e))
            stage[i] = (ao3, heads)

        def emit_attnv(i):
            ao3, heads = stage.pop(i)
            # v slots matching the score slots (aligned copies at the edges)
            va0 = v3a[:, 0:VW] if i == 0 else v3b[:, VW * (i - 1) : VW * i]
            va1 = v3a[:, VW * (NB - 1) :] if i == NB - 1 else v3b[:, VW * i : VW * (i + 1)]
            # attn @ [v | 1] for all 3 heads into one PSUM tile:
            # cols 65h..65h+63 = out_h, col 65h+64 = Z_h
            pso3 = ps_sm.tile([128, VW], F32, name=f"pso{i}", tag="sm")
            for h, e in heads:
                for c, va in enumerate((va0, va1)):
                    nc.tensor.matmul(
                        pso3[:, 65 * h : 65 * h + 65],
                        e[:, 128 * c : 128 * (c + 1)],
                        va[:, 65 * h : 65 * h + 65],
                        start=(c == 0),
                        stop=(c == 1),
                    )
            zr3 = zr_pool.tile([128, HPC], F32, name=f"zr{i}", tag="zr")
            nc.vector.reciprocal(
                zr3[:].rearrange("p (g s) -> p g s", s=1),
                pso3[:].rearrange("p (g s) -> p g s", s=65)[:, :, 64:65],
            )
            for h, _ in heads:
                nc.vector.tensor_scalar_mul(
                    ao3[:, 64 * h : 64 * (h + 1)],
                    pso3[:, 65 * h : 65 * h + 64],
                    zr3[:, h : h + 1],
                )
            # transpose [tq, 192] -> head-major [192, tq] for the out-proj
            pt0 = ps_sm.tile([128, 128], BF16, name=f"pt0_{i}", tag="sm")
            nc.tensor.transpose(pt0[:], ao3[:, 0:128], ident_sb[:])
            pt1 = ps_sm.tile([64, 128], BF16, name=f"pt1_{i}", tag="sm")
            nc.tensor.transpose(pt1[:], ao3[:, 128:192], ident_sb[:])
            nc.vector.tensor_copy(a0_sb[:, 128 * i : 128 * (i + 1)], pt0[:])
            nc.vector.tensor_copy(a1_sb[:, 128 * i : 128 * (i + 1)], pt1[:])

        def emit_outproj(n):
            # partial out-projection for tokens [512n, 512(n+1)); one
            # staging tile and ONE output DMA per quarter
            osb = o_pool.tile([128, KC * 512], F32, name=f"os{n}", tag="o")
            for dd in range(KC):
                psp = ps_big.tile([128, 512], F32, name=f"psp{n}_{dd}", tag="big")
                nc.tensor.matmul(
                    psp[:],
                    wo0_sb[:, 128 * dd : 128 * (dd + 1)],
                    a0_sb[:, 512 * n : 512 * (n + 1)],
                    start=True,
                    stop=False,
                )
                nc.tensor.matmul(
                    psp[:],
                    wo1_sb[:, 128 * dd : 128 * (dd + 1)],
                    a1_sb[:, 512 * n : 512 * (n + 1)],
                    start=False,
                    stop=True,
                )
                if dd % 2 == 0:
                    nc.scalar.copy(osb[:, 512 * dd : 512 * (dd + 1)], psp[:])
                else:
                    nc.vector.tensor_copy(osb[:, 512 * dd : 512 * (dd + 1)], psp[:])
            dst = d["outT"].rearrange("(c p) n -> p c n", c=KC)[
                :, :, 512 * n : 512 * (n + 1)
            ]
            nc.sync.dma_start(dst, osb[:].rearrange("p (c n) -> p c n", c=KC))

        for i in range(NB):
            emit_scores(i)
            if i >= PIPE:
                emit_attnv(i - PIPE)
                if (i - PIPE + 1) % 4 == 0:
                    emit_outproj((i - PIPE + 1) // 4 - 1)
        for i in range(NB - PIPE, NB):
            emit_attnv(i)
            if (i + 1) % 4 == 0:
                emit_outproj((i + 1) // 4 - 1)


_CACHED_NC = None


def build_nc():
    global _CACHED_NC
    if _CACHED_NC is not None:
        return _CACHED_NC
    nc = bass.Bass("TRN2", target_bir_lowering=False, debug=False, num_devices=NCORES)
    d = {
        "xT": nc.dram_tensor("xT", [D, N], BF16, kind="ExternalInput").ap(),
        "wqk": nc.dram_tensor("wqk", [D, 512], BF16, kind="ExternalInput").ap(),
        "wv": nc.dram_tensor("wv", [D, GC], BF16, kind="ExternalInput").ap(),
        "wo0": nc.dram_tensor("wo0", [128, D], BF16, kind="ExternalInput").ap(),
        "wo1": nc.dram_tensor("wo1", [64, D], BF16, kind="ExternalInput").ap(),
        "bqk": nc.dram_tensor("bqk", [128, 4], F32, kind="ExternalInput").ap(),
        "masks": nc.dram_tensor("masks", [128, 768], BF16, kind="ExternalInput").ap(),
        "ident": nc.dram_tensor("ident", [128, 128], BF16, kind="ExternalInput").ap(),
        "outT": nc.dram_tensor("outT", [D, N], F32, kind="ExternalOutput").ap(),
    }
    with tile.TileContext(nc) as tc:
        _emit(nc, tc, d)
    _split_multiwaits(nc)
    _CACHED_NC = nc
    return nc


def _build_masks():
    p = np.arange(128)[:, None]
    f = np.arange(128)[None, :]
    m_lo = (p >= f).astype(np.float32)                     # slot [128i-64, 128i+64)
    m_hi = (p <= f).astype(np.float32)                     # slot [128i+64, 128i+192)
    m_e0 = ((p < 64) & (p >= f - 64)).astype(np.float32)   # i=0 slot0 = [0, 128)
    m_e15 = ((p >= 64) & (p <= f + 64)).astype(np.float32)  # i=15 slot1 = [1920, 2048)
    m = np.zeros((128, 768), np.float32)
    m[:, 0:128], m[:, 128:256] = m_lo, m_hi        # variant 0: middle blocks
    m[:, 256:384], m[:, 384:512] = m_e0, m_hi      # variant 1: i = 0
    m[:, 512:640], m[:, 640:768] = m_lo, m_e15     # variant 2: i = 15
    return m.astype(ml_dtypes.bfloat16)


def make_in_maps(x, Wq, bq, Wk, bk, Wv, bv, Wo, bo):
    bf = ml_dtypes.bfloat16
    masks = _build_masks()
    ident = np.eye(128, dtype=bf)
    xT = [np.ascontiguousarray(x[b].T).astype(bf) for b in range(B)]
    in_maps = []
    pad64 = np.zeros((D, 64), np.float32)
    padb = np.zeros(64, np.float32)
    for c in range(NCORES):
        b, g = divmod(c, 4)
        s = slice(GC * g, GC * (g + 1))
        # chunks: [q0|q1], [q2|pad], [k0|k1], [k2|pad]
        wqk = np.concatenate(
            [Wq[:, s][:, :128], Wq[:, s][:, 128:], pad64,
             Wk[:, s][:, :128], Wk[:, s][:, 128:], pad64],
            axis=1,
        ).astype(bf)
        wv = np.ascontiguousarray(Wv[:, s]).astype(bf)
        wo = Wo[s, :]
        bqk = np.concatenate(
            [bq[s][:128], bq[s][128:], padb, bk[s][:128], bk[s][128:], padb]
        ).reshape(4, 128).T
        in_maps.append(
            {
                "xT": xT[b],
                "wqk": wqk,
                "wv": wv,
                "wo0": np.ascontiguousarray(wo[0:128, :]).astype(bf),
                "wo1": np.ascontiguousarray(wo[128:GC, :]).astype(bf),
                "bqk": np.ascontiguousarray(bqk, dtype=np.float32),
                "masks": masks,
                "ident": ident,
            }
        )
    return in_maps


def combine_outputs(partials, Wq, bq, Wk, bk, Wv, bv, Wo, bo):
    const = (bv.astype(np.float32) @ Wo.astype(np.float32) + bo).astype(np.float32)
    out = np.empty((B, N, D), np.float32)
    for b in range(B):
        acc = partials[4 * b].astype(np.float32).copy()
        for c in range(4 * b + 1, 4 * b + 4):
            acc += partials[c]
        out[b] = acc.T + const
    return out


def kernel(x, Wq, bq, Wk, bk, Wv, bv, Wo, bo, _trace=False, **run_kwargs):
    x = np.asarray(x, dtype=np.float32)
    args = [np.asarray(a, dtype=np.float32) for a in (Wq, bq, Wk, bk, Wv, bv, Wo, bo)]
    nc = build_nc()
    in_maps = make_in_maps(x, *args)
    res = run_bass_kernel_spmd(
        nc, in_maps, core_ids=list(range(NCORES)), trace=_trace, **run_kwargs
    )
    partials = [res.results[c]["outT"] for c in range(NCORES)]
    out = combine_outputs(partials, *args)
    if _trace:
        kernel.last_results = res
    return out


# revision 22
# speedup vs baseline: 1.2434x; 1.2434x over previous
"""DeepSeek-style local-window sparse attention on 8 TRN2 NeuronCores.

Problem: B=2, N=2048, D=768, H=12 heads x d=64, local window |q-k| <= 64,
out = softmax(mask(q k^T / 8)) v  projected by Wo.

Sharding (no on-device collectives):
  core c in 0..7 -> batch b = c//4, head group g = c%4 (heads 3g..3g+2).
  Each core computes its 3 heads' q/k/v projections over the full
  sequence, the banded attention, and a PARTIAL output projection
  (its 192 rows of Wo). The host sums the 4 partials per batch.

Device layout choices:
  - x is fed pre-transposed (xT [768, 2048]) so projections need no
    on-device transpose: q^T/k^T come out d-major (x^T as moving
    operand), v comes out token-major (x^T as stationary operand).
  - scores are computed transposed (S^T[tk, tq]) so exp(S^T) feeds the
    attn@v matmul directly as the stationary operand; the softmax
    denominator falls out of the same matmul via a ones-column
    appended to v; normalization is a per-partition scalar multiply.
  - softmax skips the running-max: scores*scale here are ~N(0, 0.31),
    so exp never overflows (verified against the reference).
  - bk shifts scores per-query-row only (softmax-invariant) but is
    applied anyway with bq via the free per-partition bias slot of the
    PSUM->SBUF copy. bv and bo pass through the softmax/projection
    linearly and are added on the host: out += bv @ Wo + bo.
  - per 128-query block i the 129-wide window is covered by exactly
    TWO 128-key slots: [128i-64, 128i+64) and [128i+64, 128i+192),
    with fixed triangular masks multiplied into exp(S^T). A second,
    64-token-shifted copy of v (built by two SBUF->SBUF DMAs — only
    DMA can move data across partitions) keeps the attn@v stationary
    operand aligned. Edge blocks swap in an aligned slot + edge mask.

Matmul operands are bf16 (f32 PSUM accumulation); partial outputs are
returned f32 and reduced on the host.
"""

import numpy as np
import ml_dtypes

import concourse.bass as bass
import concourse.tile as tile
from concourse import mybir
from concourse.bass_utils import run_bass_kernel_spmd

BF16 = mybir.dt.bfloat16
F32 = mybir.dt.float32
AF = mybir.ActivationFunctionType

B, N, D = 2, 2048, 768
H, DH = 12, 64
HPC = 3              # heads per core
GC = HPC * DH        # 192 output columns per core
NB = N // 128        # 16 query blocks
KC = D // 128        # 6 contraction chunks
SCALE = 0.125        # 1/sqrt(64)
NCORES = 8
VW = HPC * 65        # 195: v chunk width incl. ones columns


def _split_multiwaits(nc):
    """Hoist extra semaphore waits onto standalone EventSemaphore ops.

    The walrus build in this container rejects any instruction carrying
    more than one sync wait ("Too many sync wait commands"); Tile's
    semaphore assignment freely attaches several. An engine sequencer
    executes instructions in order, so waiting via a standalone
    EventSemaphore then via the instruction itself is equivalent to one
    instruction waiting on both. For DMAs the wait moves from the DGE
    descriptor to the issuing sequencer, which only delays the enqueue.
    """
    n = 0
    for fn in nc.m.functions:
        for bb in fn.blocks:
            out = []
            for inst in bb.instructions:
                si = inst.sync_info
                if si is not None and len(si.on_wait) > 1:
                    extras = list(si.on_wait[:-1])
                    si.on_wait = [si.on_wait[-1]]
                    for w in extras:
                        es = mybir.InstEventSemaphore(
                            name=f"splitw_{n}", ins=[], outs=[]
                        )
                        n += 1
                        es.engine = inst.engine
                        es.sync_info = mybir.SyncInfo(on_wait=[w], on_update=[])
                        nc.register_instruction(es)
                        out.append(es)
                out.append(inst)
            bb.instructions = out


def _emit(nc, tc, d):
    from contextlib import ExitStack

    with ExitStack() as ctx:
        const = ctx.enter_context(tc.tile_pool(name="const", bufs=1))
        persist = ctx.enter_context(tc.tile_pool(name="persist", bufs=1))
        ps_big = ctx.enter_context(tc.tile_pool(name="ps_big", bufs=3, space="PSUM"))
        ps_s = ctx.enter_context(tc.tile_pool(name="ps_s", bufs=3, space="PSUM"))
        ps_sm = ctx.enter_context(tc.tile_pool(name="ps_sm", bufs=2, space="PSUM"))
        e_pool = ctx.enter_context(tc.tile_pool(name="e_pool", bufs=10))
        ao_pool = ctx.enter_context(tc.tile_pool(name="ao_pool", bufs=3))
        o_pool = ctx.enter_context(tc.tile_pool(name="o_pool", bufs=2))
        zr_pool = ctx.enter_context(tc.tile_pool(name="zr_pool", bufs=4))

        # ---- PE warmup: dense dummy matmuls on zeroed SBUF while the
        # input DMAs land, so the HAM clock governor reaches 8/8 before
        # the first real projection instead of ~20us into them ----
        wz = const.tile([128, 512], BF16, name="wz", tag="wz")
        nc.vector.memset(wz[:], 0.0)
        psw = ps_big.tile([128, 512], F32, name="psw", tag="big")
        for _ in range(40):
            nc.tensor.matmul(psw[:], wz[:, 0:128], wz[:], start=True, stop=True)

        # ---- loads: one DMA per 128-partition chunk (parallel queues) ----
        def load_all(name, width):
            t = const.tile([128, KC * width], BF16, name=name, tag=name)
            for k in range(KC):
                nc.sync.dma_start(
                    t[:, width * k : width * (k + 1)],
                    d[name][128 * k : 128 * (k + 1), :],
                )
            return t

        xts_all = load_all("xT", N)
        wqk_all = load_all("wqk", 512)
        wv_all = load_all("wv", GC)
        xts = [xts_all[:, N * k : N * (k + 1)] for k in range(KC)]
        wqk_sb = [wqk_all[:, 512 * k : 512 * (k + 1)] for k in range(KC)]
        wv_sb = [wv_all[:, GC * k : GC * (k + 1)] for k in range(KC)]

        wo0_sb = const.tile([128, D], BF16, name="wo0", tag="wo0")
        nc.sync.dma_start(wo0_sb[:], d["wo0"][:, :])
        wo1_sb = const.tile([64, D], BF16, name="wo1", tag="wo1")
        nc.sync.dma_start(wo1_sb[:], d["wo1"][:, :])
        bqk_sb = const.tile([128, 4], F32, name="bqk", tag="bqk")
        nc.sync.dma_start(bqk_sb[:], d["bqk"][:, :])
        masks_sb = const.tile([128, 3 * 256], BF16, name="masks", tag="masks")
        nc.sync.dma_start(masks_sb[:], d["masks"][:, :])
        ident_sb = const.tile([128, 128], BF16, name="ident", tag="ident")
        nc.sync.dma_start(ident_sb[:], d["ident"][:, :])

        # ---- persistent intermediates ----
        # packed [q0|q1], [q2|pad], [k0|k1], [k2|pad] d-major: padding keeps
        # q_h and k_h at the same partition offset (matmul base-partition
        # rule); offsets 0/64 also alternate PE row groups between heads.
        qk_sb = [
            persist.tile([128, N], BF16, name=f"qk{m}", tag=f"qk{m}") for m in range(4)
        ]
        # v token-major with a ones column per head: per 128-token chunk the
        # 195 cols are [v_h0(64) | 1 | v_h1(64) | 1 | v_h2(64) | 1].
        # v3a: chunks [128j, 128j+128); v3b: shifted chunks [128j+64, ...).
        v3a = persist.tile([128, NB * VW], BF16, name="v3a", tag="v3a")
        v3b = persist.tile([128, (NB - 1) * VW], BF16, name="v3b", tag="v3b")
        ones_ap = v3a[:].rearrange("p (c g s) -> p c g s", g=HPC, s=65)[:, :, :, 64:65]
        nc.vector.memset(ones_ap, 1.0)
        # attention output, head-major (A0: packed rows 0..127, A1: 128..191)
        a0_sb = persist.tile([128, N], BF16, name="a0", tag="a0")
        a1_sb = persist.tile([64, N], BF16, name="a1", tag="a1")

        # ---- emitters ----
        def emit_qkproj(n, m):
            # qk^T chunk m for tokens [512n, 512(n+1))
            ps = ps_big.tile([128, 512], F32, name=f"psqk{m}_{n}", tag="big")
            for k in range(KC):
                nc.tensor.matmul(
                    ps[:],
                    wqk_sb[k][:, 128 * m : 128 * (m + 1)],
                    xts[k][:, 512 * n : 512 * (n + 1)],
                    start=(k == 0),
                    stop=(k == KC - 1),
                )
            nc.scalar.activation(
                qk_sb[m][:, 512 * n : 512 * (n + 1)],
                ps[:],
                AF.Identity,
                bias=bqk_sb[:, m : m + 1],
                scale=1.0,
            )

        def emit_vproj(j):
            # token-major v for chunk j: v[j] = x^T[:, chunk]^T @ wv
            psv = ps_big.tile([128, GC], F32, name=f"psv{j}", tag="big")
            for k in range(KC):
                nc.tensor.matmul(
                    psv[:],
                    xts[k][:, 128 * j : 128 * (j + 1)],
                    wv_sb[k][:],
                    start=(k == 0),
                    stop=(k == KC - 1),
                )
            vout = v3a[:, VW * j : VW * (j + 1)].rearrange(
                "p (g s) -> p g s", g=HPC, s=65
            )[:, :, 0:64]
            vin = psv[:].rearrange("p (g s) -> p g s", g=HPC, s=DH)
            nc.vector.tensor_copy(vout, vin)

        v3a_c = v3a[:].rearrange("p (c w) -> p c w", w=VW)
        v3b_c = v3b[:].rearrange("p (c w) -> p c w", w=VW)

        def emit_shift(j0, j1):
            # v3b[p, j, :] = v tokens 128j+64+p for j in [j0, j1) (partition
            # shift needs DMA). Ones columns come along for free.
            nc.gpsimd.dma_start(v3b_c[0:64, j0:j1, :], v3a_c[64:128, j0:j1, :])
            nc.gpsimd.dma_start(
                v3b_c[64:128, j0:j1, :], v3a_c[0:64, j0 + 1 : j1 + 1, :]
            )

        # ---- banded attention, software-pipelined and INTERLEAVED ----
        # Two throttle hazards: (a) PE stalls on the exp/mask chain if
        # attnV(i) directly follows scores(i) — so attnV/normalize/
        # transpose run PIPE blocks behind scores; (b) the attention
        # matmuls are small (their LDWEIGHTS dominate), and a long run of
        # them drops the PE array duty low enough that the HAM clock
        # governor falls back to half rate — so dense N=512 projection /
        # out-projection groups are woven between attention blocks.
        PIPE = 2
        stage = {}

        def emit_scores(i):
            # two 128-key slots; slot token starts (aligned at the edges)
            s0 = 0 if i == 0 else 128 * i - 64
            s1 = 128 * (NB - 1) if i == NB - 1 else 128 * i + 64
            mvar = 1 if i == 0 else (2 if i == NB - 1 else 0)
            ao3 = ao_pool.tile([128, GC], BF16, name=f"ao{i}", tag="ao")
            heads = []
            for h in range(HPC):
                # S^T[tk, tq] per slot
                pss = ps_s.tile([128, 256], F32, name=f"pss{i}_{h}", tag="s")
                mq, qo = (0, 64 * h) if h < 2 else (1, 0)
                mk, ko = (2, 64 * h) if h < 2 else (3, 0)
                for c, s in enumerate((s0, s1)):
                    nc.tensor.matmul(
                        pss[:, 128 * c : 128 * (c + 1)],
                        qk_sb[mk][ko : ko + 64, s : s + 128],
                        qk_sb[mq][qo : qo + 64, 128 * i : 128 * (i + 1)],
                        start=True,
                        stop=True,
                    )
                e = e_pool.tile([128, 256], BF16, name=f"e{i}_{h}", tag="e")
                nc.scalar.activation(e[:], pss[:], AF.Exp, scale=SCALE)
                nc.vector.tensor_mul(
                    e[:], e[:], masks_sb[:, 256 * mvar : 256 * (mvar + 1)]
                )
                heads.append((h, e))
            stage[i] = (ao3, heads)

        def emit_attnv(i):
            ao3, heads = stage.pop(i)
            # v slots matching the score slots (aligned copies at the edges)
            va0 = v3a[:, 0:VW] if i == 0 else v3b[:, VW * (i - 1) : VW * i]
            va1 = v3a[:, VW * (NB - 1) :] if i == NB - 1 else v3b[:, VW * i : VW * (i + 1)]
            # attn @ [v | 1] for all 3 heads into one PSUM tile:
            # cols 65h..65h+63 = out_h, col 65h+64 = Z_h
            pso3 = ps_sm.tile([128, VW], F32, name=f"pso{i}", tag="sm")
            for h, e in heads:
                for c, va in enumerate((va0, va1)):
                    nc.tensor.matmul(
                        pso3[:, 65 * h : 65 * h + 65],
                        e[:, 128 * c : 128 * (c + 1)],
                        va[:, 65 * h : 65 * h + 65],
                        start=(c == 0),
                        stop=(c == 1),
                    )
            zr3 = zr_pool.tile([128, HPC], F32, name=f"zr{i}", tag="zr")
            nc.vector.reciprocal(
                zr3[:].rearrange("p (g s) -> p g s", s=1),
                pso3[:].rearrange("p (g s) -> p g s", s=65)[:, :, 64:65],
            )
            for h, _ in heads:
                nc.vector.tensor_scalar_mul(
                    ao3[:, 64 * h : 64 * (h + 1)],
                    pso3[:, 65 * h : 65 * h + 64],
                    zr3[:, h : h + 1],
                )
            # transpose [tq, 192] -> head-major [192, tq] for the out-proj
            pt0 = ps_sm.tile([128, 128], BF16, name=f"pt0_{i}", tag="sm")
            nc.tensor.transpose(pt0[:], ao3[:, 0:128], ident_sb[:])
            pt1 = ps_sm.tile([64, 128], BF16, name=f"pt1_{i}", tag="sm")
            nc.tensor.transpose(pt1[:], ao3[:, 128:192], ident_sb[:])
            nc.vector.tensor_copy(a0_sb[:, 128 * i : 128 * (i + 1)], pt0[:])
            nc.vector.tensor_copy(a1_sb[:, 128 * i : 128 * (i + 1)], pt1[:])

        osb_map = {}

        def emit_outproj(n, dd):
            # one out-projection group: output rows [128dd, ...) for
            # tokens [512n, 512(n+1)); staged, ONE output DMA per quarter
            if dd == 0:
                osb_map[n] = o_pool.tile([128, KC * 512], F32, name=f"os{n}", tag="o")
            osb = osb_map[n]
            psp = ps_big.tile([128, 512], F32, name=f"psp{n}_{dd}", tag="big")
            nc.tensor.matmul(
                psp[:],
                wo0_sb[:, 128 * dd : 128 * (dd + 1)],
                a0_sb[:, 512 * n : 512 * (n + 1)],
                start=True,
                stop=False,
            )
            nc.tensor.matmul(
                psp[:],
                wo1_sb[:, 128 * dd : 128 * (dd + 1)],
                a1_sb[:, 512 * n : 512 * (n + 1)],
                start=False,
                stop=True,
            )
            if dd % 2 == 0:
                nc.scalar.copy(osb[:, 512 * dd : 512 * (dd + 1)], psp[:])
            else:
                nc.vector.tensor_copy(osb[:, 512 * dd : 512 * (dd + 1)], psp[:])
            if dd == KC - 1:
                dst = d["outT"].rearrange("(c p) n -> p c n", c=KC)[
                    :, :, 512 * n : 512 * (n + 1)
                ]
                nc.sync.dma_start(dst, osb[:].rearrange("p (c n) -> p c n", c=KC))
                del osb_map[n]

        def emit_att(i):
            emit_scores(i)
            if i >= PIPE:
                emit_attnv(i - PIPE)

        # prologue: quarter-0 projections (their first matmuls gate on the
        # input DMAs, covered by the warmup burst)
        for m in range(4):
            emit_qkproj(0, m)
        for j in range(4):
            emit_vproj(j)
        emit_shift(0, 3)

        # quarters 1..3: weave attention blocks of the previous quarter
        # between this quarter's dense projection groups
        for q in range(1, 4):
            dense = [lambda q=q, m=m: emit_qkproj(q, m) for m in range(4)]
            dense += [lambda q=q, j=j: emit_vproj(4 * q + j) for j in range(4)]
            if q >= 2:
                dense += [
                    lambda q=q, dd=dd: emit_outproj(q - 2, dd) for dd in range(KC)
                ]
            att = [4 * (q - 1) + j for j in range(4)]
            order = [dense[0], dense[1]]
            per = (len(dense) - 2) / 4.0
            used = 2.0
            for j in range(4):
                order.append(att[j])
                want = 2 + (j + 1) * per
                while used < want:
                    order.append(dense[int(used)])
                    used += 1
            for it in order:
                if callable(it):
                    it()
                else:
                    emit_att(it)
            # quarter q's v chunks enable v3b shifts through chunk 4q+2
            emit_shift(4 * q - 1, 4 * q + 3)

        # epilogue: last attention quarter woven with outproj(2), then the
        # PIPE tail and outproj(3)
        ep = [lambda dd=dd: emit_outproj(2, dd) for dd in range(KC)]
        emit_att(12)
        emit_att(13)  # flushes attnv(11) -> outproj(2) inputs complete
        ep[0]()
        ep[1]()
        emit_att(14)
        ep[2]()
        ep[3]()
        emit_att(15)
        ep[4]()
        ep[5]()
        for i in range(NB - PIPE, NB):
            emit_attnv(i)
        for dd in range(KC):
            emit_outproj(3, dd)


_CACHED_NC = None


def build_nc():
    global _CACHED_NC
    if _CACHED_NC is not None:
        return _CACHED_NC
    nc = bass.Bass("TRN2", target_bir_lowering=False, debug=False, num_devices=NCORES)
    d = {
        "xT": nc.dram_tensor("xT", [D, N], BF16, kind="ExternalInput").ap(),
        "wqk": nc.dram_tensor("wqk", [D, 512], BF16, kind="ExternalInput").ap(),
        "wv": nc.dram_tensor("wv", [D, GC], BF16, kind="ExternalInput").ap(),
        "wo0": nc.dram_tensor("wo0", [128, D], BF16, kind="ExternalInput").ap(),
        "wo1": nc.dram_tensor("wo1", [64, D], BF16, kind="ExternalInput").ap(),
        "bqk": nc.dram_tensor("bqk", [128, 4], F32, kind="ExternalInput").ap(),
        "masks": nc.dram_tensor("masks", [128, 768], BF16, kind="ExternalInput").ap(),
        "ident": nc.dram_tensor("ident", [128, 128], BF16, kind="ExternalInput").ap(),
        "outT": nc.dram_tensor("outT", [D, N], F32, kind="ExternalOutput").ap(),
    }
    with tile.TileContext(nc) as tc:
        _emit(nc, tc, d)
    _split_multiwaits(nc)
    _CACHED_NC = nc
    return nc


def _build_masks():
    p = np.arange(128)[:, None]
    f = np.arange(128)[None, :]
    m_lo = (p >= f).astype(np.float32)                     # slot [128i-64, 128i+64)
    m_hi = (p <= f).astype(np.float32)                     # slot [128i+64, 128i+192)
    m_e0 = ((p < 64) & (p >= f - 64)).astype(np.float32)   # i=0 slot0 = [0, 128)
    m_e15 = ((p >= 64) & (p <= f + 64)).astype(np.float32)  # i=15 slot1 = [1920, 2048)
    m = np.zeros((128, 768), np.float32)
    m[:, 0:128], m[:, 128:256] = m_lo, m_hi        # variant 0: middle blocks
    m[:, 256:384], m[:, 384:512] = m_e0, m_hi      # variant 1: i = 0
    m[:, 512:640], m[:, 640:768] = m_lo, m_e15     # variant 2: i = 15
    return m.astype(ml_dtypes.bfloat16)


def make_in_maps(x, Wq, bq, Wk, bk, Wv, bv, Wo, bo):
    bf = ml_dtypes.bfloat16
    masks = _build_masks()
    ident = np.eye(128, dtype=bf)
    xT = [np.ascontiguousarray(x[b].T).astype(bf) for b in range(B)]
    in_maps = []
    pad64 = np.zeros((D, 64), np.float32)
    padb = np.zeros(64, np.float32)
    for c in range(NCORES):
        b, g = divmod(c, 4)
        s = slice(GC * g, GC * (g + 1))
        # chunks: [q0|q1], [q2|pad], [k0|k1], [k2|pad]
        wqk = np.concatenate(
            [Wq[:, s][:, :128], Wq[:, s][:, 128:], pad64,
             Wk[:, s][:, :128], Wk[:, s][:, 128:], pad64],
            axis=1,
        ).astype(bf)
        wv = np.ascontiguousarray(Wv[:, s]).astype(bf)
        wo = Wo[s, :]
        bqk = np.concatenate(
            [bq[s][:128], bq[s][128:], padb, bk[s][:128], bk[s][128:], padb]
        ).reshape(4, 128).T
        in_maps.append(
            {
                "xT": xT[b],
                "wqk": wqk,
                "wv": wv,
                "wo0": np.ascontiguousarray(wo[0:128, :]).astype(bf),
                "wo1": np.ascontiguousarray(wo[128:GC, :]).astype(bf),
                "bqk": np.ascontiguousarray(bqk, dtype=np.float32),
                "masks": masks,
                "ident": ident,
            }
        )
    return in_maps


def combine_outputs(partials, Wq, bq, Wk, bk, Wv, bv, Wo, bo):
    const = (bv.astype(np.float32) @ Wo.astype(np.float32) + bo).astype(np.float32)
    out = np.empty((B, N, D), np.float32)
    for b in range(B):
        acc = partials[4 * b].astype(np.float32).copy()
        for c in range(4 * b + 1, 4 * b + 4):
            acc += partials[c]
        out[b] = acc.T + const
    return out


def kernel(x, Wq, bq, Wk, bk, Wv, bv, Wo, bo, _trace=False, **run_kwargs):
    x = np.asarray(x, dtype=np.float32)
    args = [np.asarray(a, dtype=np.float32) for a in (Wq, bq, Wk, bk, Wv, bv, Wo, bo)]
    nc = build_nc()
    in_maps = make_in_maps(x, *args)
    res = run_bass_kernel_spmd(
        nc, in_maps, core_ids=list(range(NCORES)), trace=_trace, **run_kwargs
    )
    partials = [res.results[c]["outT"] for c in range(NCORES)]
    out = combine_outputs(partials, *args)
    if _trace:
        kernel.last_results = res
    return out
